# revision 27
# baseline (speedup 1.0000x reference)
"""MoE (noisy top-2 routing, 8 experts) on 8 Trainium2 NeuronCores.

Strategy (expert-parallel, per sharding hint):
  Host: gating network (tiny: 0.1% of FLOPs) + all-to-all dispatch —
      h = x@Wg+bg + noise*softplus(x@Wn+bn), exact top-2 + softmax,
      gather each expert's tokens with capacity factor 1.0 (1024
      tokens/expert); the ~1% overflow pairs are computed exactly on
      host in fp32.
  Device (single SPMD launch, one expert per core): per-expert FFN
      y = relu(x@W1+b1)@W2 + b2   on that expert's tokens (bf16
      matmuls, fp32 PSUM accumulation, weights fully SBUF-resident).
  Host: combine — scatter-add gate-weighted per-expert outputs.
"""
import sys

sys.path.insert(0, "/opt/trn_rl_repo")
import ml_dtypes
import numpy as np
import concourse.bass as bass  # noqa: F401
from concourse import bacc
import concourse.mybir as mybir
import concourse.tile as tile
from concourse.bass_utils import run_bass_kernel_spmd
from concourse.vector_clock import ScopedClock


class _LeanTC(tile.TileContext):
    """TileContext with a lighter kernel tail: keeps the full drain (with
    waits on the global clock) and the semaphore clear (required for NEFF
    re-execution since target_bir_lowering=False has no preamble clear),
    but drops the trailing barrier — the gpsimd clear completes before
    NEFF end regardless."""

    def _drain_and_barrier(self, tick_clock, wait_clock):
        drain_inst = self.nc.sync.drain()
        wait_clock.add_sem_waits(
            drain_inst.ins, ScopedClock({None: tick_clock.global_clock}))
        self.nc.all_engine_barrier()
        popped = self.nc._tile_sem_poison_stack.pop()
        assert popped is self._sem_poison
        self.nc.clear_and_free_semaphores(
            list(self.sems.allocated().values()))

N_CORES = 8
B, S, D, H, E = 2, 2048, 768, 3072, 8
T = B * S            # 4096 tokens
KD = D // 128        # 6 contraction chunks over D
NH = H // 128        # 24 h tiles
ND = D // 128        # 6 output d tiles
CAP = 1024           # per-expert token capacity (capacity factor 1.0)
NCH = 2              # token chunks
NC = CAP // NCH      # 512 tokens per chunk (= one full PSUM bank of fp32)

F32 = mybir.dt.float32
BF16 = mybir.dt.bfloat16
AF = mybir.ActivationFunctionType
BF16NP = ml_dtypes.bfloat16

_cache = {}
last_perf = {}


def _build_ffn():
    nc = bacc.Bacc("TRN2", target_bir_lowering=False, debug=False,
                   num_devices=N_CORES)
    # weight layouts are pre-packed on host so every DMA is row-contiguous:
    #   w1 col (hh*KD + k)*128 + c  = W1[k*128+p, hh*128+c]
    #   w2 col (dt*NH + hh)*128 + c = W2[hh*128+p, dt*128+c]
    #   xc col (ch*KD + k)*NC + t   = x_tok[k*128+p, ch*NC+t]
    w1 = nc.declare_dram_parameter("w1", [128, NH * KD * 128], BF16,
                                   isOutput=False)
    w2 = nc.declare_dram_parameter("w2", [128, ND * NH * 128], BF16,
                                   isOutput=False)
    b1 = nc.declare_dram_parameter("b1", [128, NH], F32, isOutput=False)
    xc = nc.declare_dram_parameter("xc", [128, NCH * KD * NC], BF16,
                                   isOutput=False)
    yT = nc.declare_dram_parameter("yT", [D, CAP], BF16, isOutput=True)

    with _LeanTC(nc) as tc:
        with tc.tile_pool(name="sbig", bufs=1) as sbig, \
             tc.tile_pool(name="sout", bufs=4) as sout, \
             tc.tile_pool(name="psum", bufs=6, space="PSUM") as psum:
            x_sb = sbig.tile([128, NCH * KD * NC], BF16, tag="x")
            w1_sb = sbig.tile([128, NH * KD * 128], BF16, tag="w1")
            w2_sb = sbig.tile([128, ND * NH * 128], BF16, tag="w2")
            b1_sb = sbig.tile([128, NH], F32, tag="b1")
            hid_sb = sbig.tile([128, NH * NCH * NC], BF16, tag="hid")
            XW = KD * NC
            W1W = KD * 128
            W2W = NH * 128

            def w1_dma(eng, lo, hi):
                eng.dma_start(out=w1_sb[:, lo * W1W:hi * W1W],
                              in_=w1[:, lo * W1W:hi * W1W])

            # Two HWDGE rings in parallel; each DMA pays ~0.6us issue +
            # ~2us completion latency, so order exactly by first use.
            # The scalar ring gets only DMAs issued before L1's ACTIVATEs
            # are emitted (they would block later issues on that queue).
            # First matmul chain (hh=0, chunk 0) needs w1[hh0] + all of
            # x[c0]; pass order is L1(c0), L2(c0), L1(c1), L2(c1) so x[c1]
            # and w2 are only needed mid-kernel.
            # sync ring = dedicated w1 highway, DMAs aligned to the
            # hh-pair consumption order; scalar ring = everything else,
            # ordered by first-use deadline.
            for lo, hi in [(0, 2), (2, 4), (4, 6), (6, 10),
                           (10, 14), (14, 18), (18, 24)]:
                w1_dma(nc.sync, lo, hi)
            nc.scalar.dma_start(out=x_sb[:, 0:2 * NC], in_=xc[:, 0:2 * NC])
            nc.scalar.dma_start(out=x_sb[:, 2 * NC:XW], in_=xc[:, 2 * NC:XW])
            nc.scalar.dma_start(out=b1_sb[:], in_=b1[:])
            nc.scalar.dma_start(out=w2_sb[:, 0:2 * W2W], in_=w2[:, 0:2 * W2W])
            nc.scalar.dma_start(out=w2_sb[:, 2 * W2W:4 * W2W],
                                in_=w2[:, 2 * W2W:4 * W2W])
            nc.scalar.dma_start(out=x_sb[:, XW:2 * XW], in_=xc[:, XW:2 * XW])
            nc.scalar.dma_start(out=w2_sb[:, 4 * W2W:6 * W2W],
                                in_=w2[:, 4 * W2W:6 * W2W])

            # PE warmup during the initial DMA wait: junk matmuls flip the
            # HAM clock gate to full rate and absorb the first-DMA latency
            # ladder before the real work lands.
            wu_w = sout.tile([128, 128], BF16, tag="wuw")
            wu_x = sout.tile([128, NC], BF16, tag="wux")
            nc.vector.memset(wu_w[:], 0.0)
            nc.vector.memset(wu_x[:], 0.0)
            wu_ps = [psum.tile([128, NC], F32, tag="ps", name=f"wu_ps{j}")
                     for j in range(2)]
            for i in range(10):
                nc.tensor.matmul(out=wu_ps[i % 2][:], lhsT=wu_w[:],
                                 rhs=wu_x[:],
                                 start=(i < 2), stop=(i >= 8))
            nc.vector.tensor_copy(wu_x[:], wu_ps[0][:])
            nc.vector.tensor_copy(wu_x[:], wu_ps[1][:])

            # Two passes: everything for chunk 0, then chunk 1 — spreads
            # output DMAs and defers x[c1]/w2 needs to mid-kernel.
            # Within a pass, accumulation chains are interleaved in PAIRS
            # so consecutive matmuls hit different PSUM banks (same-bank
            # back-to-back serializes fill behind drain: +~45ns/matmul).
            for c in range(NCH):
                # ── layer 1: hid[hh,c] = relu(sum_k w1[k,hh].T @ x[k,c]) ──
                for hp in range(NH // 2):
                    hhs = (2 * hp, 2 * hp + 1)
                    pst = {hh: psum.tile([128, NC], F32, tag="ps",
                                         name=f"ps1_{hh}_{c}")
                           for hh in hhs}
                    for k in range(KD):
                        for hh in hhs:
                            nc.tensor.matmul(
                                out=pst[hh][:],
                                lhsT=w1_sb[:, (hh * KD + k) * 128:
                                           (hh * KD + k + 1) * 128],
                                rhs=x_sb[:, (c * KD + k) * NC:
                                         (c * KD + k + 1) * NC],
                                start=(k == 0), stop=(k == KD - 1))
                    for hh in hhs:
                        nc.scalar.activation(
                            hid_sb[:, (hh * NCH + c) * NC:
                                   (hh * NCH + c + 1) * NC],
                            pst[hh][:], AF.Relu, bias=b1_sb[:, hh:hh + 1])

                # ── layer 2: y[dt,c] = sum_hh w2[hh,dt].T @ hid[hh,c] ──
                # (b2 is applied on the host; evictions are pure copies,
                # Vector for chunk 0, Scalar for chunk 1)
                for dp in range(ND // 2):
                    dts = (2 * dp, 2 * dp + 1)
                    psy = {dt_: psum.tile([128, NC], F32, tag="ps",
                                          name=f"ps2_{dt_}_{c}")
                           for dt_ in dts}
                    for hh in range(NH):
                        for dt_ in dts:
                            nc.tensor.matmul(
                                out=psy[dt_][:],
                                lhsT=w2_sb[:, (dt_ * NH + hh) * 128:
                                           (dt_ * NH + hh + 1) * 128],
                                rhs=hid_sb[:, (hh * NCH + c) * NC:
                                           (hh * NCH + c + 1) * NC],
                                start=(hh == 0), stop=(hh == NH - 1))
                    for dt_ in dts:
                        yo = sout.tile([128, NC], BF16, tag="yo",
                                       name=f"yo_{dt_}_{c}")
                        if dt_ % 2 == 0:       # parallel eviction engines
                            nc.vector.tensor_copy(yo[:], psy[dt_][:])
                        else:
                            nc.scalar.copy(yo[:], psy[dt_][:])
                        nc.sync.dma_start(
                            out=yT[dt_ * 128:(dt_ + 1) * 128,
                                   c * NC:(c + 1) * NC],
                            in_=yo[:])
    nc.compile()
    return nc


def kernel(x, noise, Wg, bg, Wn, bn, W1, b1, W2, b2):
    x = np.asarray(x, dtype=np.float32)
    noise = np.asarray(noise, dtype=np.float32)
    Wg = np.asarray(Wg, dtype=np.float32)
    bg = np.asarray(bg, dtype=np.float32)
    Wn = np.asarray(Wn, dtype=np.float32)
    bn = np.asarray(bn, dtype=np.float32)
    W1 = np.asarray(W1, dtype=np.float32)
    b1 = np.asarray(b1, dtype=np.float32)
    W2 = np.asarray(W2, dtype=np.float32)
    b2 = np.asarray(b2, dtype=np.float32)

    if "ffn" not in _cache:
        _cache["ffn"] = _build_ffn()

    x2d = x.reshape(T, D)
    n2d = noise.reshape(T, E)

    # ── host gating: h = x@Wg+bg + noise*softplus(x@Wn+bn), exact top-2 ──
    gate = x2d @ Wg + bg
    hlog = gate + n2d * np.logaddexp(0.0, x2d @ Wn + bn)
    idx = np.argsort(-hlog, axis=1, kind="stable")[:, :2]     # [T, 2]
    vals = np.take_along_axis(hlog, idx, axis=1)
    q = np.exp(vals[:, 1] - vals[:, 0])
    p1 = 1.0 / (1.0 + q)
    probs = np.stack([p1, q * p1], axis=1).astype(np.float32)  # [T, 2]

    # ── host dispatch: gather tokens per expert (capacity CAP), pack ──
    xT = x2d.T                                                 # [D, T] view
    in_maps = []
    idxs, gates, spill = [], [], []
    for e in range(E):
        m = idx == e
        sel = np.nonzero(m.any(axis=1))[0]
        gv = np.where(m[sel, 0], probs[sel, 0], probs[sel, 1])
        if sel.size > CAP:                 # overflow pairs -> host fp32
            spill.append((e, sel[CAP:], gv[CAP:]))
            sel, gv = sel[:CAP], gv[:CAP]
        idxs.append(sel)
        gates.append(gv)
        xe = np.zeros((D, CAP), dtype=np.float32)
        xe[:, :sel.size] = xT[:, sel]
        # [k, p, ch, t] -> [p, ch, k, t]
        xp = np.ascontiguousarray(
            xe.reshape(KD, 128, NCH, NC).transpose(1, 2, 0, 3)
        ).reshape(128, NCH * KD * NC).astype(BF16NP)
        w1p = np.ascontiguousarray(
            W1[e].reshape(KD, 128, NH, 128).transpose(1, 2, 0, 3)
        ).reshape(128, NH * KD * 128).astype(BF16NP)
        w2p = np.ascontiguousarray(
            W2[e].reshape(NH, 128, ND, 128).transpose(1, 2, 0, 3)
        ).reshape(128, ND * NH * 128).astype(BF16NP)
        in_maps.append({
            "w1": w1p,
            "w2": w2p,
            "b1": np.ascontiguousarray(b1[e].reshape(NH, 128).T),
            "xc": xp,
        })

    res = run_bass_kernel_spmd(_cache["ffn"], in_maps,
                               core_ids=list(range(N_CORES)))
    last_perf["p2"] = res.exec_time_ns
    if res.instructions_and_trace:
        last_perf["p2_insts"] = res.instructions_and_trace[0]

    # ── host combine: gate-weighted scatter-add ──
    out = np.zeros((T, D), dtype=np.float32)
    for e in range(E):
        sel = idxs[e]
        yT_ = np.asarray(res.results[e]["yT"], dtype=np.float32)  # [D, CAP]
        out[sel] += (yT_[:, :sel.size].T + b2[e]) * gates[e][:, None]
    for e, sel, gv in spill:                                   # host overflow
        hid = np.maximum(x2d[sel] @ W1[e] + b1[e], 0.0)
        out[sel] += (hid @ W2[e] + b2[e]) * gv[:, None]
    return out.reshape(B, S, D)


# revision 35
# speedup vs baseline: 1.1826x; 1.1826x over previous
"""MoE (noisy top-2 routing, 8 experts) on 8 Trainium2 NeuronCores.

Strategy (expert-parallel, per sharding hint):
  Host: gating network (tiny: 0.1% of FLOPs) + all-to-all dispatch —
      h = x@Wg+bg + noise*softplus(x@Wn+bn), exact top-2 + softmax,
      gather each expert's tokens with capacity factor 1.0 (1024
      tokens/expert); the ~1% overflow pairs are computed exactly on
      host in fp32.
  Device (single SPMD launch, one expert per core): per-expert FFN
      y = relu(x@W1+b1)@W2 + b2   on that expert's tokens (bf16
      matmuls, fp32 PSUM accumulation, weights fully SBUF-resident).
  Host: combine — scatter-add gate-weighted per-expert outputs.
"""
import sys

sys.path.insert(0, "/opt/trn_rl_repo")
import ml_dtypes
import numpy as np
import concourse.bass as bass  # noqa: F401
from concourse import bacc
import concourse.mybir as mybir
import concourse.tile as tile
from concourse.bass_utils import run_bass_kernel_spmd
from concourse.vector_clock import ScopedClock


class _LeanTC(tile.TileContext):
    """TileContext with a lighter kernel tail: keeps the full drain (with
    waits on the global clock) and the semaphore clear (required for NEFF
    re-execution since target_bir_lowering=False has no preamble clear),
    but drops the trailing barrier — the gpsimd clear completes before
    NEFF end regardless."""

    def _drain_and_barrier(self, tick_clock, wait_clock):
        drain_inst = self.nc.sync.drain()
        wait_clock.add_sem_waits(
            drain_inst.ins, ScopedClock({None: tick_clock.global_clock}))
        self.nc.all_engine_barrier()
        popped = self.nc._tile_sem_poison_stack.pop()
        assert popped is self._sem_poison
        self.nc.clear_and_free_semaphores(
            list(self.sems.allocated().values()))

N_CORES = 8
B, S, D, H, E = 2, 2048, 768, 3072, 8
T = B * S            # 4096 tokens
KD = D // 128        # 6 contraction chunks over D
NH = H // 128        # 24 h tiles
ND = D // 128        # 6 output d tiles
CAP = 1024           # per-expert token capacity (capacity factor 1.0)
NCH = 2              # token chunks
NC = CAP // NCH      # 512 tokens per chunk (= one full PSUM bank of fp32)

F32 = mybir.dt.float32
BF16 = mybir.dt.bfloat16
AF = mybir.ActivationFunctionType
BF16NP = ml_dtypes.bfloat16

_cache = {}
last_perf = {}


def _build_ffn():
    nc = bacc.Bacc("TRN2", target_bir_lowering=False, debug=False,
                   num_devices=N_CORES)
    # weight layouts are pre-packed on host so every DMA is row-contiguous:
    #   w1 col (hh*KD + k)*128 + c  = W1[k*128+p, hh*128+c]
    #   w2 col (dt*NH + hh)*128 + c = W2[hh*128+p, dt*128+c]
    #   xc col (ch*KD + k)*NC + t   = x_tok[k*128+p, ch*NC+t]
    w1 = nc.declare_dram_parameter("w1", [128, NH * KD * 128], BF16,
                                   isOutput=False)
    w2 = nc.declare_dram_parameter("w2", [128, ND * NH * 128], BF16,
                                   isOutput=False)
    b1 = nc.declare_dram_parameter("b1", [128, NH], F32, isOutput=False)
    xc = nc.declare_dram_parameter("xc", [128, NCH * KD * NC], BF16,
                                   isOutput=False)
    yT = nc.declare_dram_parameter("yT", [D, CAP], BF16, isOutput=True)

    with _LeanTC(nc) as tc:
        with tc.tile_pool(name="sbig", bufs=1) as sbig, \
             tc.tile_pool(name="sout", bufs=4) as sout, \
             tc.tile_pool(name="psum", bufs=6, space="PSUM") as psum:
            x_sb = sbig.tile([128, NCH * KD * NC], BF16, tag="x")
            w1_sb = sbig.tile([128, NH * KD * 128], BF16, tag="w1")
            w2_sb = sbig.tile([128, ND * NH * 128], BF16, tag="w2")
            b1_sb = sbig.tile([128, NH], F32, tag="b1")
            hid_sb = sbig.tile([128, NH * NCH * NC], BF16, tag="hid")
            XW = KD * NC
            W1W = KD * 128
            W2W = NH * 128

            def w1_dma(eng, lo, hi):
                eng.dma_start(out=w1_sb[:, lo * W1W:hi * W1W],
                              in_=w1[:, lo * W1W:hi * W1W])

            # Two HWDGE rings in parallel; each DMA pays ~0.6us issue +
            # ~2us completion latency, so order exactly by first use.
            # The scalar ring gets only DMAs issued before L1's ACTIVATEs
            # are emitted (they would block later issues on that queue).
            # First matmul chain (hh=0, chunk 0) needs w1[hh0] + all of
            # x[c0]; pass order is L1(c0), L2(c0), L1(c1), L2(c1) so x[c1]
            # and w2 are only needed mid-kernel.
            # sync ring = dedicated w1 highway, DMAs aligned to the
            # hh-pair consumption order; scalar ring = everything else,
            # ordered by first-use deadline.
            for lo, hi in [(0, 2), (2, 4), (4, 6), (6, 10),
                           (10, 14), (14, 18), (18, 24)]:
                w1_dma(nc.sync, lo, hi)
            nc.scalar.dma_start(out=x_sb[:, 0:2 * NC], in_=xc[:, 0:2 * NC])
            nc.scalar.dma_start(out=x_sb[:, 2 * NC:XW], in_=xc[:, 2 * NC:XW])
            nc.scalar.dma_start(out=b1_sb[:], in_=b1[:])
            nc.scalar.dma_start(out=w2_sb[:, 0:2 * W2W], in_=w2[:, 0:2 * W2W])
            nc.scalar.dma_start(out=w2_sb[:, 2 * W2W:4 * W2W],
                                in_=w2[:, 2 * W2W:4 * W2W])
            nc.scalar.dma_start(out=x_sb[:, XW:2 * XW], in_=xc[:, XW:2 * XW])
            nc.scalar.dma_start(out=w2_sb[:, 4 * W2W:6 * W2W],
                                in_=w2[:, 4 * W2W:6 * W2W])

            # PE warmup during the initial DMA wait: junk matmuls flip the
            # HAM clock gate to full rate and absorb the first-DMA latency
            # ladder before the real work lands.
            wu_w = sout.tile([128, 128], BF16, tag="wuw")
            wu_x = sout.tile([128, NC], BF16, tag="wux")
            nc.vector.memset(wu_w[:], 0.0)
            nc.vector.memset(wu_x[:], 0.0)
            wu_ps = [psum.tile([128, NC], F32, tag="ps", name=f"wu_ps{j}")
                     for j in range(2)]
            for i in range(10):
                nc.tensor.matmul(out=wu_ps[i % 2][:], lhsT=wu_w[:],
                                 rhs=wu_x[:],
                                 start=(i < 2), stop=(i >= 8))
            nc.vector.tensor_copy(wu_x[:], wu_ps[0][:])
            nc.vector.tensor_copy(wu_x[:], wu_ps[1][:])

            # Two passes: everything for chunk 0, then chunk 1 — spreads
            # output DMAs and defers x[c1]/w2 needs to mid-kernel.
            # Within a pass, accumulation chains are interleaved in PAIRS
            # so consecutive matmuls hit different PSUM banks (same-bank
            # back-to-back serializes fill behind drain: +~45ns/matmul).
            for c in range(NCH):
                # ── layer 1: hid[hh,c] = relu(sum_k w1[k,hh].T @ x[k,c]) ──
                for hp in range(NH // 2):
                    hhs = (2 * hp, 2 * hp + 1)
                    pst = {hh: psum.tile([128, NC], F32, tag="ps",
                                         name=f"ps1_{hh}_{c}")
                           for hh in hhs}
                    for k in range(KD):
                        for hh in hhs:
                            nc.tensor.matmul(
                                out=pst[hh][:],
                                lhsT=w1_sb[:, (hh * KD + k) * 128:
                                           (hh * KD + k + 1) * 128],
                                rhs=x_sb[:, (c * KD + k) * NC:
                                         (c * KD + k + 1) * NC],
                                start=(k == 0), stop=(k == KD - 1))
                    for hh in hhs:
                        nc.scalar.activation(
                            hid_sb[:, (hh * NCH + c) * NC:
                                   (hh * NCH + c + 1) * NC],
                            pst[hh][:], AF.Relu, bias=b1_sb[:, hh:hh + 1])

                # ── layer 2: y[dt,c] = sum_hh w2[hh,dt].T @ hid[hh,c] ──
                # (b2 is applied on the host; evictions are pure copies,
                # Vector for chunk 0, Scalar for chunk 1)
                for dp in range(ND // 2):
                    dts = (2 * dp, 2 * dp + 1)
                    psy = {dt_: psum.tile([128, NC], F32, tag="ps",
                                          name=f"ps2_{dt_}_{c}")
                           for dt_ in dts}
                    for hh in range(NH):
                        for dt_ in dts:
                            nc.tensor.matmul(
                                out=psy[dt_][:],
                                lhsT=w2_sb[:, (dt_ * NH + hh) * 128:
                                           (dt_ * NH + hh + 1) * 128],
                                rhs=hid_sb[:, (hh * NCH + c) * NC:
                                           (hh * NCH + c + 1) * NC],
                                start=(hh == 0), stop=(hh == NH - 1))
                    for dt_ in dts:
                        yo = sout.tile([128, NC], BF16, tag="yo",
                                       name=f"yo_{dt_}_{c}")
                        if dt_ % 2 == 0:       # parallel eviction engines
                            nc.vector.tensor_copy(yo[:], psy[dt_][:])
                        else:
                            nc.scalar.copy(yo[:], psy[dt_][:])
                        nc.sync.dma_start(
                            out=yT[dt_ * 128:(dt_ + 1) * 128,
                                   c * NC:(c + 1) * NC],
                            in_=yo[:])
    nc.compile()
    return nc


def kernel(x, noise, Wg, bg, Wn, bn, W1, b1, W2, b2):
    x = np.asarray(x, dtype=np.float32)
    noise = np.asarray(noise, dtype=np.float32)
    Wg = np.asarray(Wg, dtype=np.float32)
    bg = np.asarray(bg, dtype=np.float32)
    Wn = np.asarray(Wn, dtype=np.float32)
    bn = np.asarray(bn, dtype=np.float32)
    W1 = np.asarray(W1, dtype=np.float32)
    b1 = np.asarray(b1, dtype=np.float32)
    W2 = np.asarray(W2, dtype=np.float32)
    b2 = np.asarray(b2, dtype=np.float32)

    if "ffn" not in _cache:
        _cache["ffn"] = _build_ffn()

    x2d = x.reshape(T, D)
    n2d = noise.reshape(T, E)

    # ── host gating: h = x@Wg+bg + noise*softplus(x@Wn+bn), exact top-2 ──
    gate = x2d @ Wg + bg
    hlog = gate + n2d * np.logaddexp(0.0, x2d @ Wn + bn)
    idx = np.argsort(-hlog, axis=1, kind="stable")[:, :2]     # [T, 2]
    vals = np.take_along_axis(hlog, idx, axis=1)
    q = np.exp(vals[:, 1] - vals[:, 0])
    p1 = 1.0 / (1.0 + q)
    probs = np.stack([p1, q * p1], axis=1).astype(np.float32)  # [T, 2]

    # ── host dispatch: gather tokens per expert (capacity CAP), pack ──
    xT = x2d.T                                                 # [D, T] view
    in_maps = []
    idxs, gates, spill = [], [], []
    for e in range(E):
        m = idx == e
        sel = np.nonzero(m.any(axis=1))[0]
        gv = np.where(m[sel, 0], probs[sel, 0], probs[sel, 1])
        if sel.size > CAP:                 # overflow pairs -> host fp32
            spill.append((e, sel[CAP:], gv[CAP:]))
            sel, gv = sel[:CAP], gv[:CAP]
        idxs.append(sel)
        gates.append(gv)
        xe = np.zeros((D, CAP), dtype=np.float32)
        xe[:, :sel.size] = xT[:, sel]
        # [k, p, ch, t] -> [p, ch, k, t]
        xp = np.ascontiguousarray(
            xe.reshape(KD, 128, NCH, NC).transpose(1, 2, 0, 3)
        ).reshape(128, NCH * KD * NC).astype(BF16NP)
        w1p = np.ascontiguousarray(
            W1[e].reshape(KD, 128, NH, 128).transpose(1, 2, 0, 3)
        ).reshape(128, NH * KD * 128).astype(BF16NP)
        w2p = np.ascontiguousarray(
            W2[e].reshape(NH, 128, ND, 128).transpose(1, 2, 0, 3)
        ).reshape(128, ND * NH * 128).astype(BF16NP)
        in_maps.append({
            "w1": w1p,
            "w2": w2p,
            "b1": np.ascontiguousarray(b1[e].reshape(NH, 128).T),
            "xc": xp,
        })

    res = run_bass_kernel_spmd(_cache["ffn"], in_maps,
                               core_ids=list(range(N_CORES)))
    last_perf["p2"] = res.exec_time_ns
    if res.instructions_and_trace:
        last_perf["p2_insts"] = res.instructions_and_trace[0]

    # ── host combine: gate-weighted scatter-add ──
    out = np.zeros((T, D), dtype=np.float32)
    for e in range(E):
        sel = idxs[e]
        yT_ = np.asarray(res.results[e]["yT"], dtype=np.float32)  # [D, CAP]
        out[sel] += (yT_[:, :sel.size].T + b2[e]) * gates[e][:, None]
    for e, sel, gv in spill:                                   # host overflow
        hid = np.maximum(x2d[sel] @ W1[e] + b1[e], 0.0)
        out[sel] += (hid @ W2[e] + b2[e]) * gv[:, None]
    return out.reshape(B, S, D)
